# revision 51
# baseline (speedup 1.0000x reference)
"""Megatron-style tensor-parallel causal attention (BitLinear qkv/o) on 8 TRN2 cores.

Sharding: each core owns 2 of 16 heads (qkv_weight rows) and the matching
256 o_weight columns. x/rotary replicated; partial outputs summed on host.

fp8 DoubleRow matmuls carry the projections: ternary weights are exact in
e4m3, activations are split hi/lo (x host-side, attention-out y on-device)
so each projection runs as two DoubleRow passes at half the fp16 PE cost
with ~1e-3 total error. Scores/AV/exp stay fp16 (logits span [-15, +15],
far beyond e4m3's dynamic range, so fp8 scores/probs/denominators are not
an option). PSUM is fp32 throughout. Scales are folded into data tensors
(rope carries alpha=sq/d^0.25 on both q and k; the softmax-denominator
"ones" matmul carries 1/(c*sq*so) so the final out-proj evacuation scale
is the exact constant 2^-7).

Engine budget (timeline sim): PE ~187us busy is the wall; DVE ~203us SEQ
(rope, denominator tree, evacuation halves) runs just under it; Act ~180
(exp + evacuation halves). Pool/GPSIMD cannot touch PSUM on real hardware
(walrus birverifier), so it only takes the SBUF-only y_lo split. All x
DMAs use 512-token chunks (512B contiguous runs; smaller pays 2x DMA
latency); x_lo streams after x_hi since the hi pass runs first; b1 ends
with two 256-token chunks so the last rope lands before the (1,6)/(1,7)
attention-only tail.

Causal mask is folded into the score PSUM via an identity-lhsT matmul adding
-60 to masked entries before exp. Softmax denominator uses the ones-lhsT
matmul (broadcast rows), normalization on DVE before the out-proj.
"""

import math

import numpy as np

EPS = 1e-5
NUM_HEADS = 16
HEAD_DIM = 128
B, S, H = 2, 2048, 2048
NCORES = 8
HPC = NUM_HEADS // NCORES        # heads per core = 2
FPC = 3 * HPC * HEAD_DIM         # qkv features per core = 768
P = 128
NHT = H // P                     # 16 h_in tiles
CH = 512                         # proj token chunk
NCH = S // CH                    # 4 chunks per batch
QC = 256                         # attention q chunk
NQC = S // QC                    # 8
MASKV = -60.0
SV = 1.0 / 32.0                  # v evacuation scale
# out-proj evacuation engine per (quarter*2+sub): a=Act, d=DVE
# (Pool/GPSIMD cannot read PSUM on real hardware)
OEVAC_ENG = {0: "a", 1: "d", 2: "a", 3: "d", 4: "a", 5: "d", 6: "a", 7: "d"}
CONES = 4096.0                   # ones = 1/(CONES*sq*so); out scale = 32/CONES
OUT_SCALE = 32.0 / CONES         # = 2^-7, exact


def _build_program():
    import concourse.bacc as bacc
    import concourse.mybir as mybir
    import concourse.tile as tile

    f32 = mybir.dt.float32
    f16 = mybir.dt.float16
    f8 = mybir.dt.float8e4
    AF = mybir.ActivationFunctionType
    DR = mybir.MatmulPerfMode.DoubleRow

    nc = bacc.Bacc(None, target_bir_lowering=False)

    xh = nc.dram_tensor("xh", [B, H, S], f8, kind="ExternalInput")
    xl = nc.dram_tensor("xl", [B, H, S], f8, kind="ExternalInput")
    wqkv = nc.dram_tensor("wqkv", [H, FPC], f8, kind="ExternalInput")
    wo = nc.dram_tensor("wo", [HPC * HEAD_DIM, H], f8, kind="ExternalInput")
    cos_t = nc.dram_tensor("cos_t", [P, S], f16, kind="ExternalInput")
    sin_s = nc.dram_tensor("sin_s", [P, S], f16, kind="ExternalInput")
    # aux: [0:512) mask pair (B0|B1), [512:640) identity, [640:768) ones,
    # [768] exp bias, [832:960) rotate-half permutation
    aux = nc.dram_tensor("aux", [P, 960], f16, kind="ExternalInput")
    out = nc.dram_tensor("out", [B, S, H], f16, kind="ExternalOutput")

    with tile.TileContext(nc) as tc:
        with tc.tile_pool(name="const", bufs=1) as cpool:
            # first proj chunk's x and the first weight slice lead the DMA
            # queue so the PE starts early.
            w_sb = cpool.tile([P, NHT, FPC], f8)
            wre = wqkv.rearrange("(t p) f -> p t f", p=P)
            nc.sync.dma_start(w_sb[:, 0:2, :], wre[:, 0:2, :])

            with (
                tc.tile_pool(name="qk", bufs=2) as qkpool,
                tc.tile_pool(name="vv", bufs=2) as vpool,
                tc.tile_pool(name="work", bufs=2) as wpool,
                tc.tile_pool(name="attn", bufs=3) as apool,
                tc.tile_pool(name="outp", bufs=3) as opool,
                tc.psum_pool(name="pproj", bufs=2) as pps,
                tc.psum_pool(name="pop", bufs=2) as opps,
            ):
                # uniform 512-token chunks keep every x DMA at 512B elements
                # (sub-512B runs pay 2x DMA latency); b1 ends with two 256s so
                # the (1,6) attention unit still has proj matmuls to hide under
                chunks = {
                    0: [(c * CH, CH) for c in range(NCH)],
                    1: [(0, 512), (512, 512), (1024, 512), (1536, 256), (1792, 256)],
                }

                def dma_x(dst_h, dst_l, b, t0c, W, hgrps=None):
                    for src, dst in ((xh, dst_h), (xl, dst_l)):
                        re = src[b, :, t0c : t0c + W].rearrange(
                            "(t p) c -> p t c", p=P
                        )
                        if hgrps is None:
                            nc.sync.dma_start(dst[:], re[:])
                        else:
                            for g0, g1 in hgrps:
                                nc.sync.dma_start(dst[:, g0:g1, :], re[:, g0:g1, :])

                # startup: interleave w and first-chunk x by h-group, with a
                # finer first slice so the first matmul starts ~2us in
                xh0 = wpool.tile([P, NHT, CH], f8, tag="xth")
                xl0 = wpool.tile([P, NHT, CH], f8, tag="xtl")
                xre0h = xh[0, :, 0:CH].rearrange("(t p) c -> p t c", p=P)
                xre0l = xl[0, :, 0:CH].rearrange("(t p) c -> p t c", p=P)
                # hi-pass matmuls run first, so x_lo loads after all of x_hi
                nc.sync.dma_start(xh0[:, 0:2, :], xre0h[:, 0:2, :])
                nc.sync.dma_start(w_sb[:, 2:4, :], wre[:, 2:4, :])
                nc.sync.dma_start(xh0[:, 2:4, :], xre0h[:, 2:4, :])
                for hgrp in range(1, 4):
                    nc.sync.dma_start(
                        w_sb[:, 4 * hgrp : 4 * (hgrp + 1), :],
                        wre[:, 4 * hgrp : 4 * (hgrp + 1), :],
                    )
                    nc.sync.dma_start(
                        xh0[:, 4 * hgrp : 4 * (hgrp + 1), :],
                        xre0h[:, 4 * hgrp : 4 * (hgrp + 1), :],
                    )
                nc.sync.dma_start(xl0[:, 0:8, :], xre0l[:, 0:8, :])
                nc.sync.dma_start(xl0[:, 8:16, :], xre0l[:, 8:16, :])
                # second chunk, then constants in need order: rotary halves
                # (rope of chunks 0-1), aux (first attn), out-proj weights
                xh1 = wpool.tile([P, NHT, CH], f8, tag="xth")
                xl1 = wpool.tile([P, NHT, CH], f8, tag="xtl")
                dma_x(xh1, xl1, 0, CH, CH)
                rot_sb = cpool.tile([P, 2 * S], f16)
                nc.sync.dma_start(rot_sb[:, 0:1024], cos_t[:, 0:1024])
                nc.sync.dma_start(rot_sb[:, S : S + 1024], sin_s[:, 0:1024])
                aux_sb = cpool.tile([P, 960], f16)
                nc.sync.dma_start(aux_sb[:], aux[:])
                nc.sync.dma_start(rot_sb[:, 1024:S], cos_t[:, 1024:S])
                nc.sync.dma_start(rot_sb[:, S + 1024 : 2 * S], sin_s[:, 1024:S])
                wo_sb = cpool.tile([P, HPC, H], f8)
                nc.sync.dma_start(wo_sb[:], wo.rearrange("(t p) o -> p t o", p=P))

                def late_consts():
                    pass

                msk = aux_sb[:, 0:512]          # [k,128] x (B0|B1) for diag pair
                iden = aux_sb[:, 512:640]       # identity
                ones = aux_sb[:, 640:768]       # denominator lhsT: 1/(c*sq*so)
                expb = aux_sb[:, 768:769]       # exp bias column (-8)
                perm = aux_sb[:, 832:960]       # rotate-half 64-swap permutation

                qk_raw = {}   # (b, f) -> raw (pre-rope) tiles
                qk_rope = {}  # (b, f) -> roped tiles
                v_sb = {}     # b -> v tiles [tok_part, ktile, hl*128]
                for b in range(B):
                    for f in range(4):
                        qk_raw[b, f] = qkpool.tile(
                            [P, S], f16, tag=f"qkr{f}", name=f"qkr{f}_{b}"
                        )
                        qk_rope[b, f] = qkpool.tile(
                            [P, S], f16, tag=f"qkf{f}", name=f"qkf{f}_{b}"
                        )
                    v_sb[b] = vpool.tile(
                        [P, (S // P) * 2 * P], f16, tag="v", name=f"v_{b}"
                    )

                # ---------------- projection (+rope) -----------------------
                xt_pre = {}

                def prefetch_xt(b, ci):
                    t0c, W = chunks[b][ci]
                    th = wpool.tile([P, NHT, W], f8, tag="xth", name=f"xth_{b}_{ci}")
                    tl = wpool.tile([P, NHT, W], f8, tag="xtl", name=f"xtl_{b}_{ci}")
                    dma_x(th, tl, b, t0c, W)
                    xt_pre[b, ci] = (th, tl)

                def proj_chunk(b, ci):
                    t0c, W = chunks[b][ci]
                    if b == 0 and ci == 0:
                        xh_sb, xl_sb = xh0, xl0
                    elif b == 0 and ci == 1:
                        xh_sb, xl_sb = xh1, xl1
                    elif (b, ci) in xt_pre:
                        xh_sb, xl_sb = xt_pre.pop((b, ci))
                    else:
                        xh_sb = wpool.tile(
                            [P, NHT, W], f8, tag="xth", name=f"xth_{b}_{ci}"
                        )
                        xl_sb = wpool.tile(
                            [P, NHT, W], f8, tag="xtl", name=f"xtl_{b}_{ci}"
                        )
                        dma_x(xh_sb, xl_sb, b, t0c, W)
                    # q0,q1,k0,k1 : [feat, tok] — hi pass then lo pass,
                    # DoubleRow over h-tile pairs
                    for f in range(4):
                        ps = pps.tile([P, W], f32, tag="proj", name=f"ps{b}_{ci}_{f}")
                        for xi, x_sb in enumerate((xh_sb, xl_sb)):
                            for hp in range(NHT // 2):
                                nc.tensor.matmul(
                                    ps[:],
                                    lhsT=w_sb[:, 2 * hp : 2 * hp + 2, f * P : (f + 1) * P],
                                    rhs=x_sb[:, 2 * hp : 2 * hp + 2, :],
                                    start=(xi == 0 and hp == 0),
                                    stop=(xi == 1 and hp == NHT // 2 - 1),
                                    perf_mode=DR,
                                )
                        if f % 2 == 0:
                            nc.scalar.copy(
                                qk_raw[b, f][:, t0c : t0c + W], ps[:]
                            )
                        else:
                            nc.vector.tensor_copy(
                                qk_raw[b, f][:, t0c : t0c + W], ps[:]
                            )
                    # v: [tok, feat] two tok-subs per psum tile
                    for half in range(W // 256):
                        psv = pps.tile(
                            [P, 512], f32, tag="proj", name=f"psv{b}_{ci}_{half}"
                        )
                        for sub in range(2):
                            tsub = half * 2 + sub
                            for xi, x_sb in enumerate((xh_sb, xl_sb)):
                                for hp in range(NHT // 2):
                                    nc.tensor.matmul(
                                        psv[:, sub * 2 * P : (sub + 1) * 2 * P],
                                        lhsT=x_sb[
                                            :, 2 * hp : 2 * hp + 2,
                                            tsub * P : (tsub + 1) * P,
                                        ],
                                        rhs=w_sb[:, 2 * hp : 2 * hp + 2, 4 * P : 6 * P],
                                        start=(xi == 0 and hp == 0),
                                        stop=(xi == 1 and hp == NHT // 2 - 1),
                                        perf_mode=DR,
                                    )
                        kt0 = t0c // P + half * 2
                        if half == 0:
                            nc.scalar.mul(
                                v_sb[b][:, kt0 * 2 * P : (kt0 + 2) * 2 * P], psv[:], SV
                            )
                        else:
                            nc.vector.tensor_scalar_mul(
                                v_sb[b][:, kt0 * 2 * P : (kt0 + 2) * 2 * P], psv[:], SV
                            )

                def rope_piece(b, pi, pe_swap=False):
                    # rope one proj chunk's span; runs on DVE under the next
                    # chunk's proj matmuls. pe_swap does the rotate-half on
                    # the PE (permutation matmul) instead of a DMA: ~3us of
                    # DMA fixed latency saved where rope is on the critical
                    # path (the tail, where the PE is idle anyway).
                    t0c, W = chunks[b][pi]
                    for f in range(4):
                        raw = qk_raw[b, f]
                        eng = nc.vector
                        m1 = wpool.tile(
                            [P, W], f16, tag="m1", name=f"m1{b}_{pi}_{f}"
                        )
                        eng.tensor_mul(
                            m1[:], raw[:, t0c : t0c + W], rot_sb[:, t0c : t0c + W]
                        )
                        if pe_swap:
                            qsp = opps.tile(
                                [P, W], f32, tag="op", bufs=5,
                                name=f"qsp{b}_{pi}_{f}"
                            )
                            nc.tensor.matmul(
                                qsp[:], lhsT=perm, rhs=raw[:, t0c : t0c + W],
                                start=True, stop=True,
                            )
                            qsw = wpool.tile(
                                [P, W], f16, tag="qsw", name=f"qsw{b}_{pi}_{f}"
                            )
                            nc.vector.tensor_mul(
                                qsw[:], qsp[:], rot_sb[:, S + t0c : S + t0c + W]
                            )
                        else:
                            qsw = wpool.tile(
                                [P, W], f16, tag="qsw", name=f"qsw{b}_{pi}_{f}"
                            )
                            nc.sync.dma_start(
                                qsw[0:64, :], raw[64:128, t0c : t0c + W]
                            )
                            nc.sync.dma_start(
                                qsw[64:128, :], raw[0:64, t0c : t0c + W]
                            )
                            eng.tensor_mul(
                                qsw[:], qsw[:], rot_sb[:, S + t0c : S + t0c + W]
                            )
                        eng.tensor_add(
                            qk_rope[b, f][:, t0c : t0c + W], m1[:], qsw[:]
                        )

                # ---------------- attention + out-proj ----------------------
                # The last k-tile of each q-chunk only covers q[128:256)
                # (ragged trim). Denominator: full pairs are pre-summed on DVE
                # (halves the ones-matmul rows); the ones-matmul for pair g is
                # deferred until after pair g+1's attn*v so the PE never waits
                # on the DVE add.
                def attn_unit(b, qc, hl, yph, ypl):
                    q_t = qk_rope[b, hl]
                    k_t = qk_rope[b, 2 + hl]
                    qs = q_t[:, qc * QC : (qc + 1) * QC]
                    qs_hi = q_t[:, qc * QC + P : (qc + 1) * QC]
                    yt = opps.tile([P, 512], f32, tag="op", name=f"yt{b}_{qc}_{hl}", bufs=5)
                    sm = opps.tile([P, QC], f32, tag="sum", name=f"sm{b}_{qc}_{hl}", bufs=1)
                    sum_started = False

                    def ones_mm(rhs_ap, region, stop):
                        nonlocal sum_started
                        nc.tensor.matmul(
                            sm[:, region[0] : region[1]],
                            lhsT=ones,
                            rhs=rhs_ap,
                            start=not sum_started,
                            stop=stop,
                        )
                        sum_started = True

                    def emit_scores(g):
                        diag = g == qc
                        sc = opps.tile(
                            [P, 2 * QC], f32, tag="op", bufs=5,
                            name=f"sc{b}_{qc}_{hl}_{g}",
                        )
                        nc.tensor.matmul(
                            sc[:, 0:QC],
                            lhsT=k_t[:, 2 * g * P : (2 * g + 1) * P],
                            rhs=qs,
                            start=True,
                            stop=not diag,
                        )
                        if diag:
                            # only the left [128,128] of this tile is masked
                            nc.tensor.matmul(
                                sc[:, 0:P], lhsT=iden, rhs=msk[:, 0:P],
                                start=False, stop=True,
                            )
                            nc.tensor.matmul(
                                sc[:, QC : QC + P],
                                lhsT=k_t[:, (2 * g + 1) * P : (2 * g + 2) * P],
                                rhs=qs_hi,
                                start=True,
                                stop=False,
                            )
                            nc.tensor.matmul(
                                sc[:, QC : QC + P], lhsT=iden, rhs=msk[:, 0:P],
                                start=False, stop=True,
                            )
                        else:
                            nc.tensor.matmul(
                                sc[:, QC : 2 * QC],
                                lhsT=k_t[:, (2 * g + 1) * P : (2 * g + 2) * P],
                                rhs=qs,
                                start=True,
                                stop=True,
                            )
                        return sc

                    # 3-stage pipeline: scores(g+2) and exp(g+1) run ahead of
                    # attn*v(g), so the PE never waits on the Activation
                    # engine's exp. Denominator adds (DVE) get a full
                    # iteration of slack before their ones-matmul.
                    exd = {}   # g -> (ex tile, exs tile or None)

                    def emit_exp(g):
                        nonlocal qpend, ppend
                        diag = g == qc
                        scw = 2 * QC if not diag else QC + P
                        ex = apool.tile([P, scw], f16, tag="ex")
                        nc.scalar.activation(
                            ex[:], scd[g][:, 0:scw], AF.Exp, bias=expb
                        )
                        if not diag:
                            exs = apool.tile([P, QC], f16, tag="exs", bufs=4)
                            nc.vector.tensor_add(
                                exs[:], ex[:, 0:QC], ex[:, QC : 2 * QC]
                            )
                        else:
                            # fold the whole diagonal into a leftover pending
                            # sum when one exists (its ones-matmuls vanish);
                            # otherwise keep the two 128-row matmul form
                            tgt = ppend if ppend is not None else qpend
                            if tgt is not None:
                                nc.vector.tensor_add(
                                    tgt[:, 0:P], tgt[:, 0:P], ex[:, 0:P]
                                )
                                nc.vector.tensor_add(
                                    tgt[:, P:QC], tgt[:, P:QC], ex[:, P:QC]
                                )
                                nc.vector.tensor_add(
                                    tgt[:, P:QC], tgt[:, P:QC], ex[:, QC : QC + P]
                                )
                                dmerged[0] = True
                                exs = None
                            else:
                                exs = apool.tile([P, P], f16, tag="exs", bufs=4)
                                nc.vector.tensor_add(
                                    exs[:], ex[:, P:QC], ex[:, QC : QC + P]
                                )
                        exd[g] = (ex, exs)

                    def emit_av(g):
                        nonlocal qpend, ppend, opend
                        diag = g == qc
                        scw = 2 * QC if not diag else QC + P
                        ex, exs = exd.pop(g)
                        v0 = 2 * g * 2 * P + hl * P
                        nc.tensor.matmul(
                            yt[:, 0:QC],
                            lhsT=v_sb[b][:, v0 : v0 + P],
                            rhs=ex[:, 0:QC],
                            start=(g == 0),
                            stop=False,
                        )
                        v1 = (2 * g + 1) * 2 * P + hl * P
                        nc.tensor.matmul(
                            yt[:, P:QC] if diag else yt[:, 0:QC],
                            lhsT=v_sb[b][:, v1 : v1 + P],
                            rhs=ex[:, QC:scw],
                            start=False,
                            stop=diag,
                        )
                        # quad-summed denominator: ones-matmuls run on
                        # pair-of-pair sums, each deferred one iteration so
                        # the PE never waits on the DVE adds
                        if opend is not None:
                            ones_mm(opend[:], (0, QC), stop=False)
                            opend = None
                        if not diag:
                            if qpend is None:
                                qpend = exs
                            else:
                                exq = apool.tile(
                                    [P, QC], f16, tag="exq",
                                    name=f"exq{b}_{qc}_{hl}_{g}", bufs=5,
                                )
                                nc.vector.tensor_add(exq[:], qpend[:], exs[:])
                                qpend = None
                                if ppend is None:
                                    ppend = exq
                                else:
                                    exo = apool.tile(
                                        [P, QC], f16, tag="exo",
                                        name=f"exo{b}_{qc}_{hl}_{g}",
                                    )
                                    nc.vector.tensor_add(
                                        exo[:], ppend[:], exq[:]
                                    )
                                    ppend = None
                                    opend = exo
                        elif qc == 0:
                            # no prior pair zeroed the region: cover all of it
                            ones_mm(ex[:, 0:QC], (0, QC), stop=False)
                            ones_mm(ex[:, QC : QC + P], (P, QC), stop=True)
                        elif dmerged[0]:
                            flushes = [t for t in (ppend, qpend) if t is not None]
                            for i, t in enumerate(flushes):
                                ones_mm(
                                    t[:], (0, QC), stop=(i == len(flushes) - 1)
                                )
                            ppend = qpend = None
                        else:
                            if ppend is not None:
                                ones_mm(ppend[:], (0, QC), stop=False)
                                ppend = None
                            if qpend is not None:
                                ones_mm(qpend[:], (0, QC), stop=False)
                                qpend = None
                            ones_mm(ex[:, 0:P], (0, P), stop=False)
                            ones_mm(exs[:], (P, QC), stop=True)

                    dmerged = [False]  # diag piece folded into a pending sum
                    qpend = None   # pair sum awaiting its quad partner
                    ppend = None   # quad sum awaiting its octet partner
                    opend = None   # tree sum awaiting its ones-matmul
                    scd = {0: emit_scores(0)}
                    if qc >= 1:
                        scd[1] = emit_scores(1)
                    for g in range(qc):
                        emit_exp(g)
                        if g + 2 <= qc:
                            scd[g + 2] = emit_scores(g + 2)
                        if g >= 1:
                            emit_av(g - 1)
                    if qc >= 1:
                        emit_av(qc - 1)
                    emit_exp(qc)
                    emit_av(qc)
                    recip = apool.tile([P, QC], f32, tag="rc")
                    nc.vector.reciprocal_approx_fast(recip[:], sm[:])
                    y16 = apool.tile([P, QC], f16, tag=f"y16{hl}")
                    nc.vector.tensor_mul(y16[:], yt[:, 0:QC], recip[:])
                    nc.scalar.copy(yph[:, hl, :], y16[:])
                    nc.gpsimd.tensor_sub(ypl[:, hl, :], y16[:], yph[:, hl, :])

                def oproj_part(b, qc, yph, ypl, quarters, os_sb, eng_map=None):
                    # sub-interleaved so each quarter's PSUM drain overlaps
                    # the other sub's matmuls
                    for quarter in quarters:
                        for sub in range(2):
                            ops = opps.tile([P, 512], f32, tag="op", bufs=5)
                            for yi, yp in enumerate((yph, ypl)):
                                nc.tensor.matmul(
                                    ops[:],
                                    lhsT=yp[:, :, sub * P : (sub + 1) * P],
                                    rhs=wo_sb[:, :, quarter * 512 : (quarter + 1) * 512],
                                    start=(yi == 0),
                                    stop=(yi == 1),
                                    perf_mode=DR,
                                )
                            dst = os_sb[sub][:, quarter * 512 : (quarter + 1) * 512]
                            eng = (eng_map or OEVAC_ENG)[quarter * 2 + sub]
                            if eng == "a":
                                nc.scalar.mul(dst, ops[:], OUT_SCALE)
                            elif eng == "d":
                                nc.vector.tensor_scalar_mul(dst, ops[:], OUT_SCALE)
                            else:
                                nc.gpsimd.tensor_scalar_mul(dst, ops[:], OUT_SCALE)

                def oproj_alloc(b, qc):
                    return [
                        opool.tile([P, H], f16, tag="os", name=f"os{b}_{qc}_{s}")
                        for s in range(2)
                    ]

                def oproj_flush(b, qc, os_sb, quarters):
                    # per-half flush pipelines the out DMA with evacuation
                    f0, f1 = quarters[0] * 512, (quarters[-1] + 1) * 512
                    for sub in range(2):
                        t0 = qc * QC + sub * P
                        nc.sync.dma_start(
                            out[b, t0 : t0 + P, f0:f1],
                            os_sb[sub][:, f0:f1],
                        )

                pending = None

                def attn_step(b, qc):
                    # previous chunk's out-proj lands in two half-bursts
                    # around this chunk's second head-unit: PE work that
                    # hides the normalize chain and spreads evacuations
                    nonlocal pending
                    yph = apool.tile([P, 2, QC], f8, tag="ynh", name=f"ynh{b}_{qc}")
                    ypl = apool.tile([P, 2, QC], f8, tag="ynl", name=f"ynl{b}_{qc}")
                    attn_unit(b, qc, 0, yph, ypl)
                    if pending is not None:
                        pb, pqc, pyh, pyl = pending
                        osb = oproj_alloc(pb, pqc)
                        oproj_part(pb, pqc, pyh, pyl, (0, 1), osb)
                    attn_unit(b, qc, 1, yph, ypl)
                    if pending is not None:
                        oproj_flush(pb, pqc, osb, (0, 1))
                        oproj_part(pb, pqc, pyh, pyl, (2, 3), osb)
                        oproj_flush(pb, pqc, osb, (2, 3))
                    pending = (b, qc, yph, ypl)

                # fully pipelined schedule: every attention unit runs in a
                # projection window as soon as causally possible, so the
                # exp/evac load always has proj matmuls to hide under and
                # the attention-only tail is a single qc=7 unit
                b0_units = {0: [], 1: [(0, 0), (0, 1), (0, 2), (0, 3)], 2: [(0, 4), (0, 5)],
                            3: [(0, 6)]}
                b1_units = {0: [(0, 7), (1, 0)], 1: [(1, 1), (1, 2), (1, 3)],
                            2: [(1, 4)], 3: [(1, 5)], 4: [(1, 6), (1, 7)]}
                for pi in range(len(chunks[0])):
                    proj_chunk(0, pi)
                    if pi == 0:
                        late_consts()
                    if pi + 1 < len(chunks[0]):
                        if pi + 1 >= 2:
                            prefetch_xt(0, pi + 1)
                    else:
                        prefetch_xt(1, 0)
                    rope_piece(0, pi)
                    for ub, uqc in b0_units[pi]:
                        attn_step(ub, uqc)
                for ci in range(len(chunks[1])):
                    proj_chunk(1, ci)
                    if ci + 1 < len(chunks[1]):
                        prefetch_xt(1, ci + 1)
                    rope_piece(1, ci)
                    for ub, uqc in b1_units[ci]:
                        attn_step(ub, uqc)
                pb, pqc, pyh, pyl = pending
                osb = oproj_alloc(pb, pqc)
                oproj_part(pb, pqc, pyh, pyl, (0, 1), osb)
                oproj_flush(pb, pqc, osb, (0, 1))
                oproj_part(pb, pqc, pyh, pyl, (2, 3), osb)
                oproj_flush(pb, pqc, osb, (2, 3))
    nc.finalize()
    return nc


_NC_CACHE = None


def _get_program():
    global _NC_CACHE
    if _NC_CACHE is None:
        _NC_CACHE = _build_program()
    return _NC_CACHE


def _prep_in_maps(x, rotary, qkv_weight, o_weight):
    import jax
    import ml_dtypes

    f8 = ml_dtypes.float8_e4m3fn
    bf = np.float16
    cpu = jax.devices("cpu")[0]
    with jax.default_device(cpu):
        import jax.numpy as jnp

        sq = jnp.mean(jnp.abs(jnp.asarray(qkv_weight)))
        wq_q = np.asarray(jnp.round(jnp.asarray(qkv_weight) / (sq + EPS)), np.float32)
        so = jnp.mean(jnp.abs(jnp.asarray(o_weight)))
        wo_q = np.asarray(jnp.round(jnp.asarray(o_weight) / (so + EPS)), np.float32)
        sq = float(sq)
        so = float(so)

    xt = np.ascontiguousarray(x.transpose(0, 2, 1)).astype(np.float32)
    xt_hi = xt.astype(f8)
    xt_lo = (xt - xt_hi.astype(np.float32)).astype(f8)

    # alpha on BOTH q and k rope tensors -> scores land fully scaled in PSUM
    alpha = np.float32(sq / HEAD_DIM**0.25)
    cos_t = np.ascontiguousarray(rotary[1].T * alpha).astype(bf)
    sin_t = np.ascontiguousarray(rotary[0].T * alpha).astype(np.float32)
    sin_s = sin_t.copy()
    sin_s[:64] *= -1.0
    sin_s = sin_s.astype(bf)

    # aux: mask pair for the diagonal k-tile pair, identity, ones, exp bias,
    # rotate-half permutation
    kk = np.arange(P)[:, None]
    qq = np.arange(QC)[None, :]
    aux = np.zeros((P, 960), np.float32)
    aux[:, 0:QC] = np.where(qq < kk, MASKV, 0.0)          # B0: k-tile 2qc
    aux[:, QC : 2 * QC] = np.where(qq < kk + P, MASKV, 0.0)  # B1: k-tile 2qc+1
    aux[:, 512:640] = np.eye(P)
    # denominator weight: sm = sum(exp)/(CONES*sq*so) so that the final
    # out-proj scale is the exact constant 32/CONES
    aux[:, 640:768] = 1.0 / (CONES * sq * so)
    aux[:, 768] = -8.0
    # perm[k, m] = 1 iff k == (m+64) % 128: out row m reads raw row m+64
    aux[:, 832:960] = np.eye(P)[:, np.roll(np.arange(P), -64)].T
    aux = aux.astype(bf)

    in_maps = []
    for c in range(NCORES):
        rows = []
        for part in range(3):  # q, k, v blocks of qkv_weight
            for hl in range(HPC):
                g = HPC * c + hl
                blk = wq_q[part * H + g * HEAD_DIM : part * H + (g + 1) * HEAD_DIM]
                rows.append(blk)
        wqkv_c = np.ascontiguousarray(np.concatenate(rows, axis=0).T).astype(f8)
        wo_c = np.ascontiguousarray(
            wo_q[:, c * FPC // 3 : (c + 1) * FPC // 3].T
        ).astype(f8)
        in_maps.append(
            {
                "xh": xt_hi,
                "xl": xt_lo,
                "wqkv": wqkv_c,
                "wo": wo_c,
                "cos_t": cos_t,
                "sin_s": sin_s,
                "aux": aux,
            }
        )
    return in_maps


def kernel(x, rotary, qkv_weight, o_weight):
    from concourse.bass_utils import run_bass_kernel_spmd

    in_maps = _prep_in_maps(x, rotary, qkv_weight, o_weight)
    nc = _get_program()
    res = run_bass_kernel_spmd(nc, in_maps, core_ids=list(range(NCORES)))
    acc = res.results[0]["out"].astype(np.float32)
    for c in range(1, NCORES):
        acc = acc + res.results[c]["out"].astype(np.float32)
    return acc


# revision 67
# speedup vs baseline: 1.0096x; 1.0096x over previous
"""Megatron-style tensor-parallel causal attention (BitLinear qkv/o) on 8 TRN2 cores.

Sharding: each core owns 2 of 16 heads (qkv_weight rows) and the matching
256 o_weight columns. x/rotary replicated; partial outputs summed on host.

fp8 DoubleRow matmuls carry the projections: ternary weights are exact in
e4m3, activations are split hi/lo (x host-side, attention-out y on-device)
so each projection runs as two DoubleRow passes at half the fp16 PE cost
with ~1e-3 total error. Scores/AV/exp stay fp16 (logits span [-15, +15],
far beyond e4m3's dynamic range, so fp8 scores/probs/denominators are not
an option). PSUM is fp32 throughout. Scales are folded into data tensors
(rope carries alpha=sq/d^0.25 on both q and k; the softmax-denominator
"ones" matmul carries 1/(c*sq*so) so the final out-proj evacuation scale
is the exact constant 2^-7).

Engine budget (timeline sim): PE ~187us busy is the wall; DVE ~203us SEQ
(rope, denominator tree, evacuation halves) runs just under it; Act ~180
(exp + evacuation halves). Pool/GPSIMD cannot touch PSUM on real hardware
(walrus birverifier), so it only takes the SBUF-only y_lo split. All x
DMAs use 512-token chunks (512B contiguous runs; smaller pays 2x DMA
latency); x_lo streams after x_hi since the hi pass runs first; b1 ends
with two 256-token chunks so the last rope lands before the (1,6)/(1,7)
attention-only tail.

Causal mask is folded into the score PSUM via an identity-lhsT matmul adding
-60 to masked entries before exp. Softmax denominator uses the ones-lhsT
matmul (broadcast rows), normalization on DVE before the out-proj.
"""

import math

import numpy as np

EPS = 1e-5
NUM_HEADS = 16
HEAD_DIM = 128
B, S, H = 2, 2048, 2048
NCORES = 8
HPC = NUM_HEADS // NCORES        # heads per core = 2
FPC = 3 * HPC * HEAD_DIM         # qkv features per core = 768
P = 128
NHT = H // P                     # 16 h_in tiles
CH = 512                         # proj token chunk
NCH = S // CH                    # 4 chunks per batch
QC = 256                         # attention q chunk
NQC = S // QC                    # 8
MASKV = -60.0
SV = 1.0 / 32.0                  # v evacuation scale
# out-proj evacuation engine per (quarter*2+sub): a=Act, d=DVE
# (Pool/GPSIMD cannot read PSUM on real hardware)
OEVAC_ENG = {0: "a", 1: "d", 2: "a", 3: "d", 4: "a", 5: "d", 6: "a", 7: "d"}
CONES = 4096.0                   # ones = 1/(CONES*sq*so); out scale = 32/CONES
OUT_SCALE = 32.0 / CONES         # = 2^-7, exact


def _build_program():
    import concourse.bacc as bacc
    import concourse.mybir as mybir
    import concourse.tile as tile

    f32 = mybir.dt.float32
    f16 = mybir.dt.float16
    f8 = mybir.dt.float8e4
    AF = mybir.ActivationFunctionType
    DR = mybir.MatmulPerfMode.DoubleRow

    nc = bacc.Bacc(None, target_bir_lowering=False)

    xh = nc.dram_tensor("xh", [B, H, S], f8, kind="ExternalInput")
    xl = nc.dram_tensor("xl", [B, H, S], f8, kind="ExternalInput")
    wqkv = nc.dram_tensor("wqkv", [H, FPC], f8, kind="ExternalInput")
    wo = nc.dram_tensor("wo", [HPC * HEAD_DIM, H], f8, kind="ExternalInput")
    cos_t = nc.dram_tensor("cos_t", [P, S], f16, kind="ExternalInput")
    sin_s = nc.dram_tensor("sin_s", [P, S], f16, kind="ExternalInput")
    # aux: [0:512) mask pair (B0|B1), [512:640) identity, [640:768) ones,
    # [768] exp bias, [832:960) rotate-half permutation
    aux = nc.dram_tensor("aux", [P, 960], f16, kind="ExternalInput")
    out = nc.dram_tensor("out", [B, S, H], f16, kind="ExternalOutput")

    with tile.TileContext(nc) as tc:
        with tc.tile_pool(name="const", bufs=1) as cpool:
            # first proj chunk's x and the first weight slice lead the DMA
            # queue so the PE starts early.
            w_sb = cpool.tile([P, NHT, FPC], f8)
            wre = wqkv.rearrange("(t p) f -> p t f", p=P)
            nc.sync.dma_start(w_sb[:, 0:2, :], wre[:, 0:2, :])

            with (
                tc.tile_pool(name="qk", bufs=2) as qkpool,
                tc.tile_pool(name="vv", bufs=2) as vpool,
                tc.tile_pool(name="work", bufs=2) as wpool,
                tc.tile_pool(name="attn", bufs=3) as apool,
                tc.tile_pool(name="outp", bufs=3) as opool,
                tc.psum_pool(name="pproj", bufs=2) as pps,
                tc.psum_pool(name="pop", bufs=2) as opps,
            ):
                # uniform 512-token chunks keep every x DMA at 512B elements
                # (sub-512B runs pay 2x DMA latency); b1 ends with two 256s so
                # the (1,6) attention unit still has proj matmuls to hide under
                chunks = {
                    0: [(c * CH, CH) for c in range(NCH)],
                    1: [(0, 512), (512, 512), (1024, 512), (1536, 256), (1792, 256)],
                }

                def dma_x(dst_h, dst_l, b, t0c, W, hgrps=None):
                    for src, dst in ((xh, dst_h), (xl, dst_l)):
                        re = src[b, :, t0c : t0c + W].rearrange(
                            "(t p) c -> p t c", p=P
                        )
                        if hgrps is None:
                            nc.sync.dma_start(dst[:], re[:])
                        else:
                            for g0, g1 in hgrps:
                                nc.sync.dma_start(dst[:, g0:g1, :], re[:, g0:g1, :])

                # startup: interleave w and first-chunk x by h-group, with a
                # finer first slice so the first matmul starts ~2us in
                xh0 = wpool.tile([P, NHT, CH], f8, tag="xth")
                xl0 = wpool.tile([P, NHT, CH], f8, tag="xtl")
                xre0h = xh[0, :, 0:CH].rearrange("(t p) c -> p t c", p=P)
                xre0l = xl[0, :, 0:CH].rearrange("(t p) c -> p t c", p=P)
                # hi-pass matmuls run first, so x_lo loads after all of x_hi
                nc.sync.dma_start(xh0[:, 0:2, :], xre0h[:, 0:2, :])
                nc.sync.dma_start(w_sb[:, 2:4, :], wre[:, 2:4, :])
                nc.sync.dma_start(xh0[:, 2:4, :], xre0h[:, 2:4, :])
                for hgrp in range(1, 4):
                    nc.sync.dma_start(
                        w_sb[:, 4 * hgrp : 4 * (hgrp + 1), :],
                        wre[:, 4 * hgrp : 4 * (hgrp + 1), :],
                    )
                    nc.sync.dma_start(
                        xh0[:, 4 * hgrp : 4 * (hgrp + 1), :],
                        xre0h[:, 4 * hgrp : 4 * (hgrp + 1), :],
                    )
                nc.sync.dma_start(xl0[:, 0:8, :], xre0l[:, 0:8, :])
                nc.sync.dma_start(xl0[:, 8:16, :], xre0l[:, 8:16, :])
                # second chunk, then constants in need order: rotary halves
                # (rope of chunks 0-1), aux (first attn), out-proj weights
                xh1 = wpool.tile([P, NHT, CH], f8, tag="xth")
                xl1 = wpool.tile([P, NHT, CH], f8, tag="xtl")
                dma_x(xh1, xl1, 0, CH, CH,
                      hgrps=[(0, 4), (4, 8), (8, 12), (12, 16)])
                rot_sb = cpool.tile([P, 2 * S], f16)
                nc.sync.dma_start(rot_sb[:, 0:1024], cos_t[:, 0:1024])
                nc.sync.dma_start(rot_sb[:, S : S + 1024], sin_s[:, 0:1024])
                aux_sb = cpool.tile([P, 960], f16)
                nc.sync.dma_start(aux_sb[:], aux[:])
                nc.sync.dma_start(rot_sb[:, 1024:S], cos_t[:, 1024:S])
                nc.sync.dma_start(rot_sb[:, S + 1024 : 2 * S], sin_s[:, 1024:S])
                wo_sb = cpool.tile([P, HPC, H], f8)
                nc.sync.dma_start(wo_sb[:], wo.rearrange("(t p) o -> p t o", p=P))

                def late_consts():
                    pass

                msk = aux_sb[:, 0:512]          # [k,128] x (B0|B1) for diag pair
                iden = aux_sb[:, 512:640]       # identity
                ones = aux_sb[:, 640:768]       # denominator lhsT: 1/(c*sq*so)
                expb = aux_sb[:, 768:769]       # exp bias column (-8)
                perm = aux_sb[:, 832:960]       # rotate-half 64-swap permutation

                qk_raw = {}   # (b, f) -> raw (pre-rope) tiles
                qk_rope = {}  # (b, f) -> roped tiles
                v_sb = {}     # b -> v tiles [tok_part, ktile, hl*128]
                for b in range(B):
                    for f in range(4):
                        qk_raw[b, f] = qkpool.tile(
                            [P, S], f16, tag=f"qkr{f}", name=f"qkr{f}_{b}"
                        )
                        qk_rope[b, f] = qkpool.tile(
                            [P, S], f16, tag=f"qkf{f}", name=f"qkf{f}_{b}"
                        )
                    v_sb[b] = vpool.tile(
                        [P, (S // P) * 2 * P], f16, tag="v", name=f"v_{b}"
                    )

                # ---------------- projection (+rope) -----------------------
                xt_pre = {}

                def prefetch_xt(b, ci):
                    t0c, W = chunks[b][ci]
                    th = wpool.tile([P, NHT, W], f8, tag="xth", name=f"xth_{b}_{ci}")
                    tl = wpool.tile([P, NHT, W], f8, tag="xtl", name=f"xtl_{b}_{ci}")
                    dma_x(th, tl, b, t0c, W)
                    xt_pre[b, ci] = (th, tl)

                def proj_chunk0_streaming(ci, xh_sb, xl_sb):
                    # startup chunks (0,0)/(0,1): consume each arriving w/x
                    # h-group across ALL psum groups (4 qk + 2 v, using the
                    # idle attn psum pool) so PE work per DMA group matches
                    # the feed rate
                    t0c, W = chunks[0][ci]
                    psf = [
                        opps.tile([P, W], f32, tag="op", bufs=5,
                                  name=f"ps0{ci}_{f}")
                        for f in range(4)
                    ]
                    psv = [
                        pps.tile([P, 512], f32, tag="proj", name=f"psv0{ci}_{h}")
                        for h in range(2)
                    ]
                    for xi, x_sb in enumerate((xh_sb, xl_sb)):
                        for hp in range(NHT // 2):
                            start = xi == 0 and hp == 0
                            stop = xi == 1 and hp == NHT // 2 - 1
                            for f in range(4):
                                nc.tensor.matmul(
                                    psf[f][:],
                                    lhsT=w_sb[:, 2 * hp : 2 * hp + 2, f * P : (f + 1) * P],
                                    rhs=x_sb[:, 2 * hp : 2 * hp + 2, :],
                                    start=start, stop=stop, perf_mode=DR,
                                )
                            for tsub in range(4):
                                nc.tensor.matmul(
                                    psv[tsub // 2][:, (tsub % 2) * 2 * P : (tsub % 2 + 1) * 2 * P],
                                    lhsT=x_sb[:, 2 * hp : 2 * hp + 2, tsub * P : (tsub + 1) * P],
                                    rhs=w_sb[:, 2 * hp : 2 * hp + 2, 4 * P : 6 * P],
                                    start=start, stop=stop, perf_mode=DR,
                                )
                    for f in range(4):
                        if f % 2 == 0:
                            nc.scalar.copy(qk_raw[0, f][:, t0c : t0c + W], psf[f][:])
                        else:
                            nc.vector.tensor_copy(
                                qk_raw[0, f][:, t0c : t0c + W], psf[f][:]
                            )
                    for half in range(2):
                        kt0 = t0c // P + half * 2
                        if half == 0:
                            nc.scalar.mul(
                                v_sb[0][:, kt0 * 2 * P : (kt0 + 2) * 2 * P], psv[half][:], SV
                            )
                        else:
                            nc.vector.tensor_scalar_mul(
                                v_sb[0][:, kt0 * 2 * P : (kt0 + 2) * 2 * P], psv[half][:], SV
                            )

                def proj_chunk(b, ci):
                    t0c, W = chunks[b][ci]
                    if b == 0 and ci == 0:
                        xh_sb, xl_sb = xh0, xl0
                    elif b == 0 and ci == 1:
                        xh_sb, xl_sb = xh1, xl1
                    elif (b, ci) in xt_pre:
                        xh_sb, xl_sb = xt_pre.pop((b, ci))
                    else:
                        xh_sb = wpool.tile(
                            [P, NHT, W], f8, tag="xth", name=f"xth_{b}_{ci}"
                        )
                        xl_sb = wpool.tile(
                            [P, NHT, W], f8, tag="xtl", name=f"xtl_{b}_{ci}"
                        )
                        dma_x(xh_sb, xl_sb, b, t0c, W)
                    # q0,q1,k0,k1 : [feat, tok] — hi pass then lo pass,
                    # DoubleRow over h-tile pairs
                    for f in range(4):
                        ps = pps.tile([P, W], f32, tag="proj", name=f"ps{b}_{ci}_{f}")
                        for xi, x_sb in enumerate((xh_sb, xl_sb)):
                            for hp in range(NHT // 2):
                                nc.tensor.matmul(
                                    ps[:],
                                    lhsT=w_sb[:, 2 * hp : 2 * hp + 2, f * P : (f + 1) * P],
                                    rhs=x_sb[:, 2 * hp : 2 * hp + 2, :],
                                    start=(xi == 0 and hp == 0),
                                    stop=(xi == 1 and hp == NHT // 2 - 1),
                                    perf_mode=DR,
                                )
                        if f % 2 == 0:
                            nc.scalar.copy(
                                qk_raw[b, f][:, t0c : t0c + W], ps[:]
                            )
                        else:
                            nc.vector.tensor_copy(
                                qk_raw[b, f][:, t0c : t0c + W], ps[:]
                            )
                    # v: [tok, feat] two tok-subs per psum tile
                    for half in range(W // 256):
                        psv = pps.tile(
                            [P, 512], f32, tag="proj", name=f"psv{b}_{ci}_{half}"
                        )
                        for sub in range(2):
                            tsub = half * 2 + sub
                            for xi, x_sb in enumerate((xh_sb, xl_sb)):
                                for hp in range(NHT // 2):
                                    nc.tensor.matmul(
                                        psv[:, sub * 2 * P : (sub + 1) * 2 * P],
                                        lhsT=x_sb[
                                            :, 2 * hp : 2 * hp + 2,
                                            tsub * P : (tsub + 1) * P,
                                        ],
                                        rhs=w_sb[:, 2 * hp : 2 * hp + 2, 4 * P : 6 * P],
                                        start=(xi == 0 and hp == 0),
                                        stop=(xi == 1 and hp == NHT // 2 - 1),
                                        perf_mode=DR,
                                    )
                        kt0 = t0c // P + half * 2
                        if half == 0:
                            nc.scalar.mul(
                                v_sb[b][:, kt0 * 2 * P : (kt0 + 2) * 2 * P], psv[:], SV
                            )
                        else:
                            nc.vector.tensor_scalar_mul(
                                v_sb[b][:, kt0 * 2 * P : (kt0 + 2) * 2 * P], psv[:], SV
                            )

                def rope_piece(b, pi, pe_swap=False):
                    # rope one proj chunk's span; runs on DVE under the next
                    # chunk's proj matmuls. pe_swap does the rotate-half on
                    # the PE (permutation matmul) instead of a DMA: ~3us of
                    # DMA fixed latency saved where rope is on the critical
                    # path (the tail, where the PE is idle anyway).
                    t0c, W = chunks[b][pi]
                    for f in range(4):
                        raw = qk_raw[b, f]
                        eng = nc.vector
                        m1 = wpool.tile(
                            [P, W], f16, tag="m1", name=f"m1{b}_{pi}_{f}"
                        )
                        eng.tensor_mul(
                            m1[:], raw[:, t0c : t0c + W], rot_sb[:, t0c : t0c + W]
                        )
                        if pe_swap:
                            qsp = opps.tile(
                                [P, W], f32, tag="op", bufs=5,
                                name=f"qsp{b}_{pi}_{f}"
                            )
                            nc.tensor.matmul(
                                qsp[:], lhsT=perm, rhs=raw[:, t0c : t0c + W],
                                start=True, stop=True,
                            )
                            qsw = wpool.tile(
                                [P, W], f16, tag="qsw", name=f"qsw{b}_{pi}_{f}"
                            )
                            nc.vector.tensor_mul(
                                qsw[:], qsp[:], rot_sb[:, S + t0c : S + t0c + W]
                            )
                        else:
                            qsw = wpool.tile(
                                [P, W], f16, tag="qsw", name=f"qsw{b}_{pi}_{f}"
                            )
                            nc.sync.dma_start(
                                qsw[0:64, :], raw[64:128, t0c : t0c + W]
                            )
                            nc.sync.dma_start(
                                qsw[64:128, :], raw[0:64, t0c : t0c + W]
                            )
                            eng.tensor_mul(
                                qsw[:], qsw[:], rot_sb[:, S + t0c : S + t0c + W]
                            )
                        eng.tensor_add(
                            qk_rope[b, f][:, t0c : t0c + W], m1[:], qsw[:]
                        )

                # ---------------- attention + out-proj ----------------------
                # The last k-tile of each q-chunk only covers q[128:256)
                # (ragged trim). Denominator: full pairs are pre-summed on DVE
                # (halves the ones-matmul rows); the ones-matmul for pair g is
                # deferred until after pair g+1's attn*v so the PE never waits
                # on the DVE add.
                def attn_unit(b, qc, hl, yph, ypl):
                    q_t = qk_rope[b, hl]
                    k_t = qk_rope[b, 2 + hl]
                    qs = q_t[:, qc * QC : (qc + 1) * QC]
                    qs_hi = q_t[:, qc * QC + P : (qc + 1) * QC]
                    yt = opps.tile([P, 512], f32, tag="op", name=f"yt{b}_{qc}_{hl}", bufs=5)
                    sm = opps.tile([P, QC], f32, tag="sum", name=f"sm{b}_{qc}_{hl}", bufs=1)
                    sum_started = False

                    def ones_mm(rhs_ap, region, stop):
                        nonlocal sum_started
                        nc.tensor.matmul(
                            sm[:, region[0] : region[1]],
                            lhsT=ones,
                            rhs=rhs_ap,
                            start=not sum_started,
                            stop=stop,
                        )
                        sum_started = True

                    def emit_scores(g):
                        diag = g == qc
                        sc = opps.tile(
                            [P, 2 * QC], f32, tag="op", bufs=5,
                            name=f"sc{b}_{qc}_{hl}_{g}",
                        )
                        nc.tensor.matmul(
                            sc[:, 0:QC],
                            lhsT=k_t[:, 2 * g * P : (2 * g + 1) * P],
                            rhs=qs,
                            start=True,
                            stop=not diag,
                        )
                        if diag:
                            # only the left [128,128] of this tile is masked
                            nc.tensor.matmul(
                                sc[:, 0:P], lhsT=iden, rhs=msk[:, 0:P],
                                start=False, stop=True,
                            )
                            nc.tensor.matmul(
                                sc[:, QC : QC + P],
                                lhsT=k_t[:, (2 * g + 1) * P : (2 * g + 2) * P],
                                rhs=qs_hi,
                                start=True,
                                stop=False,
                            )
                            nc.tensor.matmul(
                                sc[:, QC : QC + P], lhsT=iden, rhs=msk[:, 0:P],
                                start=False, stop=True,
                            )
                        else:
                            nc.tensor.matmul(
                                sc[:, QC : 2 * QC],
                                lhsT=k_t[:, (2 * g + 1) * P : (2 * g + 2) * P],
                                rhs=qs,
                                start=True,
                                stop=True,
                            )
                        return sc

                    # 3-stage pipeline: scores(g+2) and exp(g+1) run ahead of
                    # attn*v(g), so the PE never waits on the Activation
                    # engine's exp. Denominator adds (DVE) get a full
                    # iteration of slack before their ones-matmul.
                    exd = {}   # g -> (ex tile, exs tile or None)

                    def emit_exp(g):
                        nonlocal qpend, ppend
                        diag = g == qc
                        scw = 2 * QC if not diag else QC + P
                        ex = apool.tile([P, scw], f16, tag="ex")
                        nc.scalar.activation(
                            ex[:], scd[g][:, 0:scw], AF.Exp, bias=expb
                        )
                        if not diag:
                            exs = apool.tile([P, QC], f16, tag="exs", bufs=4)
                            nc.vector.tensor_add(
                                exs[:], ex[:, 0:QC], ex[:, QC : 2 * QC]
                            )
                        else:
                            # fold the whole diagonal into a leftover pending
                            # sum when one exists (its ones-matmuls vanish);
                            # otherwise keep the two 128-row matmul form
                            tgt = ppend if ppend is not None else qpend
                            if tgt is not None:
                                nc.vector.tensor_add(
                                    tgt[:, 0:P], tgt[:, 0:P], ex[:, 0:P]
                                )
                                nc.vector.tensor_add(
                                    tgt[:, P:QC], tgt[:, P:QC], ex[:, P:QC]
                                )
                                nc.vector.tensor_add(
                                    tgt[:, P:QC], tgt[:, P:QC], ex[:, QC : QC + P]
                                )
                                dmerged[0] = True
                                exs = None
                            else:
                                exs = apool.tile([P, P], f16, tag="exs", bufs=4)
                                nc.vector.tensor_add(
                                    exs[:], ex[:, P:QC], ex[:, QC : QC + P]
                                )
                        exd[g] = (ex, exs)

                    def emit_av(g):
                        nonlocal qpend, ppend, opend
                        diag = g == qc
                        scw = 2 * QC if not diag else QC + P
                        ex, exs = exd.pop(g)
                        v0 = 2 * g * 2 * P + hl * P
                        nc.tensor.matmul(
                            yt[:, 0:QC],
                            lhsT=v_sb[b][:, v0 : v0 + P],
                            rhs=ex[:, 0:QC],
                            start=(g == 0),
                            stop=False,
                        )
                        v1 = (2 * g + 1) * 2 * P + hl * P
                        nc.tensor.matmul(
                            yt[:, P:QC] if diag else yt[:, 0:QC],
                            lhsT=v_sb[b][:, v1 : v1 + P],
                            rhs=ex[:, QC:scw],
                            start=False,
                            stop=diag,
                        )
                        # quad-summed denominator: ones-matmuls run on
                        # pair-of-pair sums, each deferred one iteration so
                        # the PE never waits on the DVE adds
                        if opend is not None:
                            ones_mm(opend[:], (0, QC), stop=False)
                            opend = None
                        if not diag:
                            if qpend is None:
                                qpend = exs
                            else:
                                exq = apool.tile(
                                    [P, QC], f16, tag="exq",
                                    name=f"exq{b}_{qc}_{hl}_{g}", bufs=5,
                                )
                                nc.vector.tensor_add(exq[:], qpend[:], exs[:])
                                qpend = None
                                if ppend is None:
                                    ppend = exq
                                else:
                                    exo = apool.tile(
                                        [P, QC], f16, tag="exo",
                                        name=f"exo{b}_{qc}_{hl}_{g}",
                                    )
                                    nc.vector.tensor_add(
                                        exo[:], ppend[:], exq[:]
                                    )
                                    ppend = None
                                    opend = exo
                        elif qc == 0:
                            # no prior pair zeroed the region: cover all of it
                            ones_mm(ex[:, 0:QC], (0, QC), stop=False)
                            ones_mm(ex[:, QC : QC + P], (P, QC), stop=True)
                        elif dmerged[0]:
                            flushes = [t for t in (ppend, qpend) if t is not None]
                            for i, t in enumerate(flushes):
                                ones_mm(
                                    t[:], (0, QC), stop=(i == len(flushes) - 1)
                                )
                            ppend = qpend = None
                        else:
                            if ppend is not None:
                                ones_mm(ppend[:], (0, QC), stop=False)
                                ppend = None
                            if qpend is not None:
                                ones_mm(qpend[:], (0, QC), stop=False)
                                qpend = None
                            ones_mm(ex[:, 0:P], (0, P), stop=False)
                            ones_mm(exs[:], (P, QC), stop=True)

                    dmerged = [False]  # diag piece folded into a pending sum
                    qpend = None   # pair sum awaiting its quad partner
                    ppend = None   # quad sum awaiting its octet partner
                    opend = None   # tree sum awaiting its ones-matmul
                    scd = {0: emit_scores(0)}
                    if qc >= 1:
                        scd[1] = emit_scores(1)
                    for g in range(qc):
                        emit_exp(g)
                        if g + 2 <= qc:
                            scd[g + 2] = emit_scores(g + 2)
                        if g >= 1:
                            emit_av(g - 1)
                    if qc >= 1:
                        emit_av(qc - 1)
                    emit_exp(qc)
                    emit_av(qc)
                    recip = apool.tile([P, QC], f32, tag="rc")
                    nc.vector.reciprocal_approx_fast(recip[:], sm[:])
                    y16 = apool.tile([P, QC], f16, tag=f"y16{hl}")
                    nc.vector.tensor_mul(y16[:], yt[:, 0:QC], recip[:])
                    if b == 1 and qc >= 6:
                        # tail units are Act-exp-bound: keep y_hi off Act
                        nc.vector.tensor_copy(yph[:, hl, :], y16[:])
                    else:
                        nc.scalar.copy(yph[:, hl, :], y16[:])
                    nc.gpsimd.tensor_sub(ypl[:, hl, :], y16[:], yph[:, hl, :])

                def oproj_part(b, qc, yph, ypl, quarters, os_sb, eng_map=None):
                    # sub-interleaved so each quarter's PSUM drain overlaps
                    # the other sub's matmuls
                    for quarter in quarters:
                        for sub in range(2):
                            ops = opps.tile([P, 512], f32, tag="op", bufs=5)
                            for yi, yp in enumerate((yph, ypl)):
                                nc.tensor.matmul(
                                    ops[:],
                                    lhsT=yp[:, :, sub * P : (sub + 1) * P],
                                    rhs=wo_sb[:, :, quarter * 512 : (quarter + 1) * 512],
                                    start=(yi == 0),
                                    stop=(yi == 1),
                                    perf_mode=DR,
                                )
                            dst = os_sb[sub][:, quarter * 512 : (quarter + 1) * 512]
                            eng = (eng_map or OEVAC_ENG)[quarter * 2 + sub]
                            if eng == "a":
                                nc.scalar.mul(dst, ops[:], OUT_SCALE)
                            elif eng == "d":
                                nc.vector.tensor_scalar_mul(dst, ops[:], OUT_SCALE)
                            else:
                                nc.gpsimd.tensor_scalar_mul(dst, ops[:], OUT_SCALE)

                def oproj_alloc(b, qc):
                    return [
                        opool.tile([P, H], f16, tag="os", name=f"os{b}_{qc}_{s}")
                        for s in range(2)
                    ]

                def oproj_flush(b, qc, os_sb, quarters):
                    # per-half flush pipelines the out DMA with evacuation
                    f0, f1 = quarters[0] * 512, (quarters[-1] + 1) * 512
                    for sub in range(2):
                        t0 = qc * QC + sub * P
                        nc.sync.dma_start(
                            out[b, t0 : t0 + P, f0:f1],
                            os_sb[sub][:, f0:f1],
                        )

                pending = None

                def attn_step(b, qc):
                    # previous chunk's out-proj lands in two half-bursts
                    # around this chunk's second head-unit: PE work that
                    # hides the normalize chain and spreads evacuations
                    nonlocal pending
                    yph = apool.tile([P, 2, QC], f8, tag="ynh", name=f"ynh{b}_{qc}")
                    ypl = apool.tile([P, 2, QC], f8, tag="ynl", name=f"ynl{b}_{qc}")
                    emap = None
                    attn_unit(b, qc, 0, yph, ypl)
                    if pending is not None:
                        pb, pqc, pyh, pyl = pending
                        osb = oproj_alloc(pb, pqc)
                        oproj_part(pb, pqc, pyh, pyl, (0, 1), osb, eng_map=emap)
                    attn_unit(b, qc, 1, yph, ypl)
                    if pending is not None:
                        oproj_flush(pb, pqc, osb, (0, 1))
                        oproj_part(pb, pqc, pyh, pyl, (2, 3), osb, eng_map=emap)
                        oproj_flush(pb, pqc, osb, (2, 3))
                    pending = (b, qc, yph, ypl)

                # fully pipelined schedule: every attention unit runs in a
                # projection window as soon as causally possible, so the
                # exp/evac load always has proj matmuls to hide under and
                # the attention-only tail is a single qc=7 unit
                b0_units = {0: [], 1: [(0, 0), (0, 1), (0, 2), (0, 3)], 2: [(0, 4), (0, 5)],
                            3: [(0, 6)]}
                b1_units = {0: [(0, 7), (1, 0)], 1: [(1, 1), (1, 2), (1, 3)],
                            2: [(1, 4)], 3: [(1, 5)], 4: [(1, 6), (1, 7)]}
                for pi in range(len(chunks[0])):
                    proj_chunk(0, pi)
                    if pi == 0:
                        late_consts()
                    if pi + 1 < len(chunks[0]):
                        if pi + 1 >= 2:
                            prefetch_xt(0, pi + 1)
                    else:
                        prefetch_xt(1, 0)
                    rope_piece(0, pi)
                    for ub, uqc in b0_units[pi]:
                        attn_step(ub, uqc)
                for ci in range(len(chunks[1])):
                    proj_chunk(1, ci)
                    if ci + 1 < len(chunks[1]):
                        prefetch_xt(1, ci + 1)
                    rope_piece(1, ci)
                    for ub, uqc in b1_units[ci]:
                        attn_step(ub, uqc)
                pb, pqc, pyh, pyl = pending
                osb = oproj_alloc(pb, pqc)
                oproj_part(pb, pqc, pyh, pyl, (0, 1), osb)
                oproj_flush(pb, pqc, osb, (0, 1))
                oproj_part(pb, pqc, pyh, pyl, (2, 3), osb)
                oproj_flush(pb, pqc, osb, (2, 3))
    nc.finalize()
    return nc


_NC_CACHE = None


def _get_program():
    global _NC_CACHE
    if _NC_CACHE is None:
        _NC_CACHE = _build_program()
    return _NC_CACHE


def _prep_in_maps(x, rotary, qkv_weight, o_weight):
    import jax
    import ml_dtypes

    f8 = ml_dtypes.float8_e4m3fn
    bf = np.float16
    cpu = jax.devices("cpu")[0]
    with jax.default_device(cpu):
        import jax.numpy as jnp

        sq = jnp.mean(jnp.abs(jnp.asarray(qkv_weight)))
        wq_q = np.asarray(jnp.round(jnp.asarray(qkv_weight) / (sq + EPS)), np.float32)
        so = jnp.mean(jnp.abs(jnp.asarray(o_weight)))
        wo_q = np.asarray(jnp.round(jnp.asarray(o_weight) / (so + EPS)), np.float32)
        sq = float(sq)
        so = float(so)

    xt = np.ascontiguousarray(x.transpose(0, 2, 1)).astype(np.float32)
    xt_hi = xt.astype(f8)
    xt_lo = (xt - xt_hi.astype(np.float32)).astype(f8)

    # alpha on BOTH q and k rope tensors -> scores land fully scaled in PSUM
    alpha = np.float32(sq / HEAD_DIM**0.25)
    cos_t = np.ascontiguousarray(rotary[1].T * alpha).astype(bf)
    sin_t = np.ascontiguousarray(rotary[0].T * alpha).astype(np.float32)
    sin_s = sin_t.copy()
    sin_s[:64] *= -1.0
    sin_s = sin_s.astype(bf)

    # aux: mask pair for the diagonal k-tile pair, identity, ones, exp bias,
    # rotate-half permutation
    kk = np.arange(P)[:, None]
    qq = np.arange(QC)[None, :]
    aux = np.zeros((P, 960), np.float32)
    aux[:, 0:QC] = np.where(qq < kk, MASKV, 0.0)          # B0: k-tile 2qc
    aux[:, QC : 2 * QC] = np.where(qq < kk + P, MASKV, 0.0)  # B1: k-tile 2qc+1
    aux[:, 512:640] = np.eye(P)
    # denominator weight: sm = sum(exp)/(CONES*sq*so) so that the final
    # out-proj scale is the exact constant 32/CONES
    aux[:, 640:768] = 1.0 / (CONES * sq * so)
    aux[:, 768] = -8.0
    # perm[k, m] = 1 iff k == (m+64) % 128: out row m reads raw row m+64
    aux[:, 832:960] = np.eye(P)[:, np.roll(np.arange(P), -64)].T
    aux = aux.astype(bf)

    in_maps = []
    for c in range(NCORES):
        rows = []
        for part in range(3):  # q, k, v blocks of qkv_weight
            for hl in range(HPC):
                g = HPC * c + hl
                blk = wq_q[part * H + g * HEAD_DIM : part * H + (g + 1) * HEAD_DIM]
                rows.append(blk)
        wqkv_c = np.ascontiguousarray(np.concatenate(rows, axis=0).T).astype(f8)
        wo_c = np.ascontiguousarray(
            wo_q[:, c * FPC // 3 : (c + 1) * FPC // 3].T
        ).astype(f8)
        in_maps.append(
            {
                "xh": xt_hi,
                "xl": xt_lo,
                "wqkv": wqkv_c,
                "wo": wo_c,
                "cos_t": cos_t,
                "sin_s": sin_s,
                "aux": aux,
            }
        )
    return in_maps


def kernel(x, rotary, qkv_weight, o_weight):
    from concourse.bass_utils import run_bass_kernel_spmd

    in_maps = _prep_in_maps(x, rotary, qkv_weight, o_weight)
    nc = _get_program()
    res = run_bass_kernel_spmd(nc, in_maps, core_ids=list(range(NCORES)))
    acc = res.results[0]["out"].astype(np.float32)
    for c in range(1, NCORES):
        acc = acc + res.results[c]["out"].astype(np.float32)
    return acc
